# revision 17
# baseline (speedup 1.0000x reference)
"""Binarized MLP (784 -> 1024 -> 1024 -> 1024 -> 10) on 8 TRN2 NeuronCores.

Data-parallel over the batch (16384 rows -> 2048 per core), weights replicated.

Math notes (these make the kernel both fast and numerically faithful):
  * Layers 1-2 outputs are only ever consumed through binarize(hardtanh(bn(h))).
    Since hardtanh preserves sign and bn here is (h - m) * rsqrt(v+eps) * g + be
    with g > 0, be == 0, the next-layer input is exactly sign(h + (b - m)).
    That is one ScalarE Sign activation with a per-partition bias, no bn needed.
  * fc2/fc3 multiply two +-1 operands -> exact in fp8(e4m3) with fp32 PSUM
    accumulation (integer partial sums, magnitude <= 1024). DoubleRow perf mode
    contracts two chunks per pass (2 fp8 weights per PE cell).
  * fc1 splits x = hi + lo with hi = fp16(x), lo = e4m3((x - hi) * 2^13).
    hi rides 7 fp16 passes (112 rows each, 784 = 7*112, no tail pass); lo
    rides 4 fp8 DoubleRow passes (8 chunks of 98 rows). Both operands are
    pre-scaled by 2^13 (exact, power of two) so the tiny lo digits clear the
    e4m3 subnormal floor (2^-9) and BOTH passes accumulate into the SAME
    PSUM group; sign() is scale-invariant so the 2^13 folds into the sign
    bias. 11 passes instead of 13 for ~16.5 significand bits of x, which
    keeps the end-to-end rel err ~1e-2 (verified bit-exactly offline on the
    deterministic inputs).
  * fc4 + log_softmax: logits computed feature-major [10, B] in PSUM, DVE-
    transposed to [B, 10] straight from PSUM (no copy hop); b4 broadcast-added
    on DVE post-transpose; log_softmax without max-subtraction (logits small).

Loop order: weights stationary per (m, k); all 4 batch column chunks stream
per weight load (amortizes LDWEIGHTS). 4 PSUM banks accumulate per m-tile,
8-slot pool double-buffers across m-tiles.

Schedule/latency techniques (vs the naive pipeline):
  * dummy-matmul warmup bridges the NEFF preamble to first-data so the PE's
    HAM clock gate is at 8/8 (2.4 GHz) when fc1 starts (~5us).
  * fc1 m=0/m=1 interleave over k: every arriving x chunk feeds 8 queued MMs,
    so the PE FIFO never blocks on a not-yet-arrived chunk (~5us).
  * x rides the two HWDGE rings as FULL-chunk DMAs (4KB per-partition lines;
    small strided descriptors measured ~95GB/s/ring vs ~250+ for big ones);
    first hi chunk is split in halves so the first MMs start ~1.3us earlier.
  * all weight/bias DRAM layouts host-pretransposed to partition-major so
    every DMA is contiguous; s1 m>=2 staggered behind fc1 progress on the
    SWDGE ring; s2/s3 deferred behind the x load.
  * fc4 processed in 5 pieces (512x3 + 256x2) with psum copies + exp on
    ScalarE, transpose/reduce on DVE: the post-last-MM chain is short.
  * ln(sum(exp)) computed table-free via exponent/mantissa bit tricks on
    DVE + one ScalarE uint32 convert, dodging the 1.3us Ln ACT_TABLE_LOAD
    (Ln is outside the boot-resident table group; Exp/Identity are inside).
  * y stored in the SBUF-contiguous [32, blk, 10] layout (host un-permutes);
    the row-major layout costs 2000+ 40-byte DMA descriptors.
"""

import os
import numpy as np

N_CORES = 8
B_FULL = 16384
BS = B_FULL // N_CORES  # 2048 rows per core
IN_F = 784
HIC = 7                 # fc1 hi chunks of 112 rows (7*112 = 784)
HIR = 112
LOP = 4                 # fc1 lo DoubleRow pairs (8 chunks of 98 rows)
LOR = 98
XSC = 8192.0            # 2^13 pre-scale on both fc1 operand digits
H = 1024
HC = 8                  # hidden chunks of 128
OUT_F = 10
NSPLIT = 4              # batch column chunks of 512
NB = BS // NSPLIT       # 512
BT = BS // 128          # 16 batch tiles of 128 for the output transpose

LAST_RESULT = None      # BassKernelResults of the most recent run (for test.py)

_PLAN = {}


def _build_nc():
    import concourse.bass as bass
    import concourse.mybir as mybir
    import concourse.tile as tile
    from concourse.tile import add_dep_helper
    from concourse import bacc
    from concourse.bass import ts
    from concourse.masks import make_identity

    f32 = mybir.dt.float32
    f16 = mybir.dt.float16
    f8 = mybir.dt.float8e4
    AF = mybir.ActivationFunctionType
    ALU = mybir.AluOpType
    DR = mybir.MatmulPerfMode.DoubleRow

    nc = bacc.Bacc(None)

    xhi_t = nc.dram_tensor("xhi", [HIC, HIR, BS], f16, kind="ExternalInput")
    xlo_t = nc.dram_tensor("xlo", [LOP, LOR, 2, BS], f8, kind="ExternalInput")
    # weight layouts are host-pretransposed to [m, p, k, c] so every DMA is
    # a contiguous per-partition stream (no tiny strided descriptors)
    s1h_t = nc.dram_tensor("s1h", [HC, HIR, HIC, 128], f16, kind="ExternalInput")
    s1l_t = nc.dram_tensor("s1l", [HC, LOR, LOP, 2, 128], f8, kind="ExternalInput")
    s2_t = nc.dram_tensor("s2t", [HC, 128, HC, 128], f8, kind="ExternalInput")
    s3_t = nc.dram_tensor("s3t", [HC, 128, HC, 128], f8, kind="ExternalInput")
    w4_t = nc.dram_tensor("w4t", [128, HC, OUT_F], f16, kind="ExternalInput")
    b1_t = nc.dram_tensor("bias1", [128, HC], f32, kind="ExternalInput")
    b2_t = nc.dram_tensor("bias2", [128, HC], f32, kind="ExternalInput")
    sc3_t = nc.dram_tensor("sc3", [128, HC], f32, kind="ExternalInput")
    sh3_t = nc.dram_tensor("sh3", [128, HC], f32, kind="ExternalInput")
    b4_t = nc.dram_tensor("b4", [32, OUT_F], f32, kind="ExternalInput")
    # y stays in the SBUF-contiguous [32, blk, 10] layout; the host
    # un-permutes (free) instead of paying a 40B-per-row strided DMA
    y_t = nc.dram_tensor("y", [32, BS // 32, OUT_F], f32, kind="ExternalOutput")

    with tile.TileContext(nc) as tc:
        with (
            tc.tile_pool(name="consts", bufs=1) as consts,
            tc.tile_pool(name="tmp", bufs=4) as tmp,
            tc.tile_pool(name="psum", bufs=8, space="PSUM") as psum,
        ):
            junk16 = consts.tile([128, 512], f16, tag="junk16")
            xh_sb = consts.tile([HIR, HIC, BS], f16, tag="xh")
            xl_sb = consts.tile([LOR, LOP, 2, BS], f8, tag="xl")
            s1h_sb = consts.tile([HIR, HC, HIC, 128], f16, tag="s1h")
            s1l_sb = consts.tile([LOR, HC, LOP, 2, 128], f8, tag="s1l")
            s2_sb = consts.tile([128, HC, HC, 128], f8, tag="s2")
            s3_sb = consts.tile([128, HC, HC, 128], f8, tag="s3")
            w4_sb = consts.tile([128, HC, OUT_F], f16, tag="w4")
            b1v = consts.tile([128, HC], f32, tag="b1v")
            b2v = consts.tile([128, HC], f32, tag="b2v")
            sc3v = consts.tile([128, HC], f32, tag="sc3v")
            sh3v = consts.tile([128, HC], f32, tag="sh3v")
            b4v = consts.tile([32, OUT_F], f32, tag="b4v")
            act1 = consts.tile([128, HC, BS], f8, tag="act1")
            act2 = consts.tile([128, HC, BS], f8, tag="act2")
            act3 = consts.tile([128, HC, BS], f16, tag="act3")
            NBLK = BS // 32  # 64 batch blocks of 32 for the DVE transpose
            ltr = consts.tile([32, BS], f32, tag="ltr")
            es2 = consts.tile([32, NBLK, OUT_F], f32, tag="es2")
            lse2 = consts.tile([32, NBLK], f32, tag="lse2")
            outf2 = consts.tile([32, NBLK, OUT_F], f32, tag="outf2")
            u32 = mybir.dt.uint32
            lnmu = consts.tile([32, NBLK], u32, tag="lnmu")
            lnt = consts.tile([32, NBLK], f32, tag="lnt")
            lnbf = consts.tile([32, NBLK], f32, tag="lnbf")
            ksc = consts.tile([32, 1], f32, tag="ksc")
            kbi = consts.tile([32, 1], f32, tag="kbi")

            # ---- PE warmup: dummy matmuls on zeroed junk keep the PE busy
            # from the end of the NEFF preamble until real data arrives, so
            # the HAM clock-gate is at 8/8 (2.4 GHz) when fc1 starts and no
            # ramp time is wasted.
            # memset on gpsimd: it reaches body code ~0.8us before vector,
            # so the warmup (and the HAM SHORT window) starts that much sooner
            nc.gpsimd.memset(junk16, 0.0)
            wps = [psum.tile([128, NB], f32, tag="mm", name="wps") for _ in range(2)]
            # sized so the HAM SHORT window fires (needs a CONTIGUOUS ~3.4us
            # busy stretch covering a full free-running 4096-cycle window)
            # before the first data-paced stall can re-throttle the ramp
            for i in range(12):
                nc.tensor.matmul(wps[i % 2], junk16[:, 0:128], junk16,
                                 start=True, stop=True)
            for i in range(4):
                nc.tensor.matmul(wps[i % 2][:, 0:128], junk16[:, 0:128],
                                 junk16[:, 0:128], start=True, stop=True)

            # ---- input DMAs. Per-DMA-queue throughput is ~95-130GB/s
            # shared among in-flight transfers (2 HWDGE rings + 1 SWDGE,
            # ~260GB/s aggregate), so x streams in consumption order with
            # ~230KB granularity. Weight heads are split small so the
            # first LDWEIGHTS doesn't wait on a 200KB transfer.
            nc.sync.dma_start(out=s1h_sb[:, 0, 0:2], in_=s1h_t[0, :, 0:2])
            nc.scalar.dma_start(out=s1h_sb[:, 1, 0:2], in_=s1h_t[1, :, 0:2])
            nc.scalar.dma_start(out=b1v, in_=b1_t[:, :])
            # s1h chunk 2+ for m=0/1 ride SWDGE (needed only ~2 units in) so
            # the HW ring heads reach the first x bytes sooner
            nc.gpsimd.dma_start(out=s1h_sb[:, 0, 2:HIC], in_=s1h_t[0, :, 2:HIC])
            nc.gpsimd.dma_start(out=s1h_sb[:, 1, 2:HIC], in_=s1h_t[1, :, 2:HIC])
            # hi chunks as 230KB halves, one per HW ring per unit, in
            # consumption order; lo pair 0 rides the SWDGE ring (dep-free,
            # ahead of all dep-gated weight loads), pairs 1-3 the HW rings
            for u in range(HIC):
                if u < 2:
                    # first two units at quarter granularity: the (u, n) MMs
                    # wait on per-DMA completion sems, so finer pieces let
                    # the first MMs fire ~1us earlier at identical ring load
                    for n in range(2):
                        nc.sync.dma_start(out=xh_sb[:, u, ts(n, NB)],
                                          in_=xhi_t[u, :, ts(n, NB)])
                        nc.scalar.dma_start(out=xh_sb[:, u, ts(n + 2, NB)],
                                            in_=xhi_t[u, :, ts(n + 2, NB)])
                else:
                    nc.sync.dma_start(out=xh_sb[:, u, 0:NB * 2],
                                      in_=xhi_t[u, :, 0:NB * 2])
                    nc.scalar.dma_start(out=xh_sb[:, u, NB * 2:BS],
                                        in_=xhi_t[u, :, NB * 2:BS])
            nc.gpsimd.dma_start(out=s1l_sb[:, 0], in_=s1l_t[0])
            nc.gpsimd.dma_start(out=s1l_sb[:, 1], in_=s1l_t[1])
            nc.gpsimd.dma_start(out=xl_sb[:, 0], in_=xlo_t[0])
            for p in (1, 2, 3):
                nc.sync.dma_start(out=xl_sb[:, p, :, 0:NB * 2],
                                  in_=xlo_t[p, :, :, 0:NB * 2])
                nc.scalar.dma_start(out=xl_sb[:, p, :, NB * 2:BS],
                                    in_=xlo_t[p, :, :, NB * 2:BS])
            # fc1 weights for m>=2: SWDGE, staggered behind fc1 progress
            s1_dmas = {}
            for m in range(2, HC):
                dh = nc.gpsimd.dma_start(out=s1h_sb[:, m], in_=s1h_t[m])
                dl = nc.gpsimd.dma_start(out=s1l_sb[:, m], in_=s1l_t[m])
                s1_dmas[m] = (dh, dl)

            # ---- fc1: h1 = x.T @ s1 (feature-major), sign -> act1 ----
            # units 0-6: fp16 hi chunks (112 rows); units 7-10: fp8
            # DoubleRow lo pairs (2x98 rows). All accumulate into one PSUM
            # group at 2^13 scale; sign ignores the scale (bias pre-scaled).
            # m=0 and m=1 interleave over units so every arriving x chunk
            # feeds 8 MMs -- the PE never queues a not-yet-arrived chunk's
            # MM ahead of runnable work during the x DMA window.
            NU = HIC + LOP  # 11 units
            psp = {m: [psum.tile([128, NB], f32, tag="mm", name="ps")
                       for _ in range(NSPLIT)] for m in (0, 1)}
            for u in range(NU):
                for m in (0, 1):
                    for n in range(NSPLIT):
                        if u < HIC:
                            nc.tensor.matmul(
                                psp[m][n], s1h_sb[:, m, u], xh_sb[:, u, ts(n, NB)],
                                start=(u == 0), stop=(u == NU - 1),
                            )
                        else:
                            p = u - HIC
                            nc.tensor.matmul(
                                psp[m][n], s1l_sb[:, m, p], xl_sb[:, p, :, ts(n, NB)],
                                start=False, stop=(u == NU - 1),
                                perf_mode=DR,
                            )
            for m in (0, 1):
                for n in range(NSPLIT):
                    a = nc.scalar.activation(
                        act1[:, m, ts(n, NB)], psp[m][n], AF.Sign, bias=b1v[:, m:m + 1]
                    )
                    if m == 1 and n == NSPLIT - 1:
                        x_done_gate = a
                    if n == 0 and m + 3 in s1_dmas:
                        for d in s1_dmas[m + 3]:
                            add_dep_helper(d.ins, a.ins, reason="stagger s1 loads")
            for m in range(2, HC):
                pss = [psum.tile([128, NB], f32, tag="mm", name="ps") for _ in range(NSPLIT)]
                for u in range(NU):
                    for n in range(NSPLIT):
                        if u < HIC:
                            nc.tensor.matmul(
                                pss[n], s1h_sb[:, m, u], xh_sb[:, u, ts(n, NB)],
                                start=(u == 0), stop=(u == NU - 1),
                            )
                        else:
                            p = u - HIC
                            nc.tensor.matmul(
                                pss[n], s1l_sb[:, m, p], xl_sb[:, p, :, ts(n, NB)],
                                start=False, stop=(u == NU - 1),
                                perf_mode=DR,
                            )
                for n in range(NSPLIT):
                    a = nc.scalar.activation(
                        act1[:, m, ts(n, NB)], pss[n], AF.Sign, bias=b1v[:, m:m + 1]
                    )
                    # stagger s1 weight loads two m-tiles ahead of use
                    if n == 0 and m + 3 in s1_dmas:
                        for d in s1_dmas[m + 3]:
                            add_dep_helper(d.ins, a.ins, reason="stagger s1 loads")

            # later-layer weights: gated behind fc1 m=1 so their transfers
            # don't steal HBM bandwidth from the x load during the ramp.
            # s2 rides the (now idle) sync HWDGE ring; s3 the SWDGE ring.
            for m in range(HC):
                d = nc.sync.dma_start(out=s2_sb[:, m], in_=s2_t[m])
                add_dep_helper(d.ins, x_done_gate.ins, reason="defer s2 after x load")
            nc.scalar.dma_start(out=b2v, in_=b2_t[:, :])
            for m in range(HC):
                d = nc.gpsimd.dma_start(out=s3_sb[:, m], in_=s3_t[m])
                add_dep_helper(d.ins, x_done_gate.ins, reason="defer s3 after x load")
            nc.scalar.dma_start(out=sc3v, in_=sc3_t[:, :])
            nc.scalar.dma_start(out=sh3v, in_=sh3_t[:, :])
            nc.scalar.dma_start(out=w4_sb, in_=w4_t[:, :, :])
            nc.scalar.dma_start(out=b4v, in_=b4_t[:, :])
            nc.vector.memset(ksc, 8.262958294867817e-08)    # ln2 * 2^-23
            nc.vector.memset(kbi, -88.81559010855409)       # -127*ln2 + ln2*c0

            # ---- fc2: binary x binary, fp8 DoubleRow, sign -> act2 ----
            # half-tiles (2 psum banks per group): the ScalarE psum-drain
            # vents every 2 n-chunks so bank recycling never stalls the PE
            for m in range(HC):
                for hf in range(2):
                    pss = [psum.tile([128, NB], f32, tag="mm", name="ps")
                           for _ in range(2)]
                    for kk in range(HC // 2):
                        ksl = slice(2 * kk, 2 * kk + 2)
                        for j in range(2):
                            n = 2 * hf + j
                            nc.tensor.matmul(
                                pss[j], s2_sb[:, m, ksl], act1[:, ksl, ts(n, NB)],
                                start=(kk == 0), stop=(kk == HC // 2 - 1),
                                perf_mode=DR,
                            )
                    for j in range(2):
                        n = 2 * hf + j
                        nc.scalar.activation(
                            act2[:, m, ts(n, NB)], pss[j], AF.Sign,
                            bias=b2v[:, m:m + 1]
                        )

            # ---- fc3: fp8 DoubleRow, bn affine + hardtanh -> act3 (DVE) ----
            for m in range(HC):
                for hf in range(2):
                    pss = [psum.tile([128, NB], f32, tag="mm", name="ps")
                           for _ in range(2)]
                    for kk in range(HC // 2):
                        ksl = slice(2 * kk, 2 * kk + 2)
                        for j in range(2):
                            n = 2 * hf + j
                            nc.tensor.matmul(
                                pss[j], s3_sb[:, m, ksl], act2[:, ksl, ts(n, NB)],
                                start=(kk == 0), stop=(kk == HC // 2 - 1),
                                perf_mode=DR,
                            )
                    for j in range(2):
                        n = 2 * hf + j
                        t = tmp.tile([128, NB], f32, tag="t3")
                        nc.scalar.activation(
                            t, pss[j], AF.Identity,
                            bias=sh3v[:, m:m + 1], scale=sc3v[:, m:m + 1],
                        )
                        nc.vector.tensor_scalar(
                            out=act3[:, m, ts(n, NB)], in0=t,
                            scalar1=-1.0, scalar2=1.0,
                            op0=ALU.max, op1=ALU.min,
                        )

            # ---- fc4 + log_softmax + store, streamed per 512-col n-chunk
            # so only the last chunk's (short) chain trails the final MM.
            # ltr[p, 32j+q] = logit class q of batch row 32j+p  (q < 10)
            # chunk widths: the final pieces are small so the post-last-MM
            # chain (copy/transpose/bias/exp/reduce) is short
            C3, C2, C1 = 0.10668443736693474, -0.7135874857296783, 1.3937265161635166

            def fin(bs, be, ring):
                # table-free ln(lse) for 32-blocks [bs:be), then subtract and
                # store those blocks. ln(s) = ln2*(float(bits)*2^-23 - 127
                # + c0) + poly3(m), m = mantissa in [1,2); dodges the 1.28us
                # Ln ACT_TABLE_LOAD (Ln is outside the boot-resident group).
                bsl = slice(bs, be)
                bits = lse2[:, bsl].bitcast(u32)
                nc.scalar.activation(lnbf[:, bsl], bits, AF.Identity,
                                     scale=ksc[:, :], bias=kbi[:, :])
                nc.vector.tensor_scalar(out=lnmu[:, bsl], in0=bits,
                                        scalar1=0x007FFFFF, op0=ALU.bitwise_and,
                                        scalar2=0x3F800000, op1=ALU.bitwise_or)
                mant = lnmu[:, bsl].bitcast(f32)
                nc.vector.tensor_scalar(out=lnt[:, bsl], in0=mant, scalar1=C3,
                                        op0=ALU.mult, scalar2=C2, op1=ALU.add)
                nc.vector.tensor_tensor(out=lnt[:, bsl], in0=lnt[:, bsl],
                                        in1=mant, op=ALU.mult)
                nc.vector.tensor_scalar(out=lnt[:, bsl], in0=lnt[:, bsl],
                                        scalar1=C1, op0=ALU.add, scalar2=None)
                nc.vector.tensor_tensor(out=lnt[:, bsl], in0=lnt[:, bsl],
                                        in1=mant, op=ALU.mult)
                nc.vector.tensor_tensor(out=lse2[:, bsl], in0=lnt[:, bsl],
                                        in1=lnbf[:, bsl], op=ALU.add)
                ltf = ltr[:, bs * 32:be * 32]
                ltvf = bass.AP(tensor=ltf.tensor, offset=ltf.offset,
                               ap=[ltf.ap[0], [32, be - bs], [1, OUT_F]])
                lser = lse2[:, bsl]
                nc.vector.tensor_tensor(
                    out=outf2[:, bsl], in0=ltvf,
                    in1=bass.AP(tensor=lser.tensor, offset=lser.offset,
                                ap=[lser.ap[0], lser.ap[1], [0, OUT_F]]),
                    op=ALU.subtract,
                )
                ring.dma_start(out=y_t[:, bsl], in_=outf2[:, bsl])

            pieces = [(0, NB), (NB, NB), (2 * NB, NB), (3 * NB, NB // 2),
                      (3 * NB + NB // 2, NB // 4), (3 * NB + 3 * NB // 4, NB // 4)]
            for pi, (st, w) in enumerate(pieces):
                wb = w // 32
                sb = st // 32
                csl = slice(st, st + w)
                bsl = slice(sb, sb + wb)
                # psum [32, w] so the DVE transpose can read it directly
                # (rows 10-31 junk, never consumed); b4 added post-transpose
                # on DVE (hidden engine slack) instead of costing PE columns
                ps4 = psum.tile([32, w], f32, tag="mm", name="ps4")
                for k in range(HC):
                    nc.tensor.matmul(
                        ps4[0:OUT_F, :], w4_sb[:, k], act3[:, k, csl],
                        start=(k == 0), stop=(k == HC - 1),
                    )
                nc.vector.transpose(ltr[:, csl], ps4[:, :])
                ltrn = ltr[:, csl]
                ltv = bass.AP(tensor=ltrn.tensor, offset=ltrn.offset,
                              ap=[ltrn.ap[0], [32, wb], [1, OUT_F]])
                b4r = b4v[:, :]
                nc.vector.tensor_tensor(
                    out=ltv, in0=ltv,
                    in1=bass.AP(tensor=b4r.tensor, offset=b4r.offset,
                                ap=[b4r.ap[0], [0, wb], b4r.ap[1]]),
                    op=ALU.add,
                )
                nc.scalar.activation(es2[:, bsl], ltv, AF.Exp)
                nc.vector.tensor_reduce(
                    out=lse2[:, bsl], in_=es2[:, bsl],
                    axis=mybir.AxisListType.X, op=ALU.add,
                )
                # finalize blocks 0-47 while pieces 3-5 are still on the PE,
                # blocks 48-59 during piece 5; only 4 blocks trail the last MM
                if pi == 2:
                    fin(0, 48, nc.scalar)
                elif pi == 4:
                    fin(48, 60, nc.scalar)
            fin(60, NBLK, nc.sync)

    nc.finalize()
    return nc


def _host_prep(inputs):
    """Shard x, binarize/lay out weights, fold bn into sign biases."""
    import ml_dtypes

    f16 = np.float16
    f8 = ml_dtypes.float8_e4m3

    x = np.asarray(inputs["x"], np.float32)
    w1 = np.asarray(inputs["w1"], np.float32)
    w2 = np.asarray(inputs["w2"], np.float32)
    w3 = np.asarray(inputs["w3"], np.float32)
    w4 = np.asarray(inputs["w4"], np.float32)
    b1 = np.asarray(inputs["b1"], np.float32)
    b2 = np.asarray(inputs["b2"], np.float32)
    b3 = np.asarray(inputs["b3"], np.float32)
    b4 = np.asarray(inputs["b4"], np.float32)

    EPS = np.float64(1e-5)

    def gv(i):
        return (np.asarray(inputs[f"g{i}"], np.float32),
                np.asarray(inputs[f"be{i}"], np.float32),
                np.asarray(inputs[f"m{i}"], np.float32),
                np.asarray(inputs[f"v{i}"], np.float32))

    g1, be1, m1, v1 = gv(1)
    g2, be2, m2, v2 = gv(2)
    g3, be3, m3, v3 = gv(3)
    # sign(bn(h)) == sign(h + (b - m)) requires gamma > 0 and beta == 0
    assert np.all(g1 > 0) and np.all(be1 == 0), "unsupported bn1 params"
    assert np.all(g2 > 0) and np.all(be2 == 0), "unsupported bn2 params"

    def pvec(v):  # [H] -> [128, HC] partition-major
        return np.ascontiguousarray(v.astype(np.float32).reshape(HC, 128).T)

    bias1 = pvec(b1 - m1) * np.float32(XSC)  # fc1 psum carries 2^13 * h1
    bias2 = pvec(b2 - m2)
    r3 = 1.0 / np.sqrt(v3.astype(np.float64) + EPS)
    sc3 = pvec(r3 * g3)
    sh3 = pvec((b3 - m3).astype(np.float64) * r3 * g3 + be3)

    def wlay(w, kc, dt):  # [out, in] -> [m, 128p(in), k, 128c(out)]
        st = np.sign(w).T.astype(np.float32)            # [in, out]
        kin = kc * 128
        if st.shape[0] < kin:
            st = np.pad(st, ((0, kin - st.shape[0]), (0, 0)))
        mo = st.shape[1] // 128
        return np.ascontiguousarray(
            st.reshape(kc, 128, mo, 128).transpose(2, 1, 0, 3)
        ).astype(dt)

    s1f = np.sign(w1).T.astype(np.float32)              # [784, 1024]
    # hi: [8m, 112, 7, 128] fp16; lo: [8m, 98, 4, 2, 128] fp8 (DR pairs)
    s1h = np.ascontiguousarray(
        s1f.reshape(HIC, HIR, HC, 128).transpose(2, 1, 0, 3)
    ).astype(f16)
    s1l = np.ascontiguousarray(
        s1f.reshape(2 * LOP, LOR, HC, 128).transpose(2, 1, 0, 3)
        .reshape(HC, LOR, LOP, 2, 128)
    ).astype(f8)
    s2t = wlay(w2, HC, f8)
    s3t = wlay(w3, HC, f8)
    w4t = np.ascontiguousarray(
        w4.T.astype(f16).reshape(HC, 128, OUT_F).transpose(1, 0, 2)
    )

    b4p = np.ascontiguousarray(np.broadcast_to(b4.reshape(1, OUT_F), (32, OUT_F)).astype(np.float32))
    shared = dict(s1h=s1h, s1l=s1l, s2t=s2t, s3t=s3t, w4t=w4t,
                  bias1=bias1, bias2=bias2, sc3=sc3, sh3=sh3, b4=b4p)
    in_maps = []
    for c in range(N_CORES):
        xs = x[c * BS:(c + 1) * BS]                     # [2048, 784]
        xt = xs.T                                       # [784, 2048]
        xhi = xt.astype(f16)
        xlo = xt - xhi.astype(np.float32)
        # both digits pre-scaled by 2^13 (exact): hi stays fp16-exact
        # (max |x|*8192 ~ 45k < 65504), lo clears the e4m3 subnormal floor
        xhi_s = (xhi.astype(np.float32) * np.float32(XSC)).astype(f16)
        xlo_s = (xlo * np.float32(XSC)).astype(f8)
        m = dict(shared)
        m["xhi"] = np.ascontiguousarray(xhi_s.reshape(HIC, HIR, BS))
        m["xlo"] = np.ascontiguousarray(
            xlo_s.reshape(LOP, 2, LOR, BS).transpose(0, 2, 1, 3))
        in_maps.append(m)
    return in_maps


def kernel(**inputs):
    global LAST_RESULT
    from concourse.bass_utils import run_bass_kernel_spmd

    if "nc" not in _PLAN:
        _PLAN["nc"] = _build_nc()
    nc = _PLAN["nc"]

    in_maps = _host_prep(inputs)
    br = run_bass_kernel_spmd(
        nc, in_maps, list(range(N_CORES)),
        tmpdir=os.environ.get("KERNEL_TMPDIR") or None,
    )
    LAST_RESULT = br
    out = np.concatenate(
        [br.results[c]["y"].transpose(1, 0, 2).reshape(BS, OUT_F)
         for c in range(N_CORES)], axis=0)
    return out.astype(np.float32)


# revision 18
# speedup vs baseline: 1.0114x; 1.0114x over previous
"""Binarized MLP (784 -> 1024 -> 1024 -> 1024 -> 10) on 8 TRN2 NeuronCores.

Data-parallel over the batch (16384 rows -> 2048 per core), weights replicated.

Math notes (these make the kernel both fast and numerically faithful):
  * Layers 1-2 outputs are only ever consumed through binarize(hardtanh(bn(h))).
    Since hardtanh preserves sign and bn here is (h - m) * rsqrt(v+eps) * g + be
    with g > 0, be == 0, the next-layer input is exactly sign(h + (b - m)).
    That is one ScalarE Sign activation with a per-partition bias, no bn needed.
  * fc2/fc3 multiply two +-1 operands -> exact in fp8(e4m3) with fp32 PSUM
    accumulation (integer partial sums, magnitude <= 1024). DoubleRow perf mode
    contracts two chunks per pass (2 fp8 weights per PE cell).
  * fc1 splits x = hi + lo with hi = fp16(x), lo = e4m3((x - hi) * 2^13).
    hi rides 7 fp16 passes (112 rows each, 784 = 7*112, no tail pass); lo
    rides 4 fp8 DoubleRow passes (8 chunks of 98 rows). Both operands are
    pre-scaled by 2^13 (exact, power of two) so the tiny lo digits clear the
    e4m3 subnormal floor (2^-9) and BOTH passes accumulate into the SAME
    PSUM group; sign() is scale-invariant so the 2^13 folds into the sign
    bias. 11 passes instead of 13 for ~16.5 significand bits of x, which
    keeps the end-to-end rel err ~1e-2 (verified bit-exactly offline on the
    deterministic inputs).
  * fc4 + log_softmax: logits computed feature-major [10, B] in PSUM, DVE-
    transposed to [B, 10] straight from PSUM (no copy hop); b4 broadcast-added
    on DVE post-transpose; log_softmax without max-subtraction (logits small).

Loop order: weights stationary per (m, k); all 4 batch column chunks stream
per weight load (amortizes LDWEIGHTS). 4 PSUM banks accumulate per m-tile,
8-slot pool double-buffers across m-tiles.

Schedule/latency techniques (vs the naive pipeline):
  * dummy-matmul warmup bridges the NEFF preamble to first-data so the PE's
    HAM clock gate is at 8/8 (2.4 GHz) when fc1 starts (~5us).
  * fc1 m=0/m=1 interleave over k: every arriving x chunk feeds 8 queued MMs,
    so the PE FIFO never blocks on a not-yet-arrived chunk (~5us).
  * x rides the two HWDGE rings as FULL-chunk DMAs (4KB per-partition lines;
    small strided descriptors measured ~95GB/s/ring vs ~250+ for big ones);
    first hi chunk is split in halves so the first MMs start ~1.3us earlier.
  * all weight/bias DRAM layouts host-pretransposed to partition-major so
    every DMA is contiguous; s1 m>=2 staggered behind fc1 progress on the
    SWDGE ring; s2/s3 deferred behind the x load.
  * fc4 processed in 5 pieces (512x3 + 256x2) with psum copies + exp on
    ScalarE, transpose/reduce on DVE: the post-last-MM chain is short.
  * ln(sum(exp)) computed table-free via exponent/mantissa bit tricks on
    DVE + one ScalarE uint32 convert, dodging the 1.3us Ln ACT_TABLE_LOAD
    (Ln is outside the boot-resident table group; Exp/Identity are inside).
  * y stored in the SBUF-contiguous [32, blk, 10] layout (host un-permutes);
    the row-major layout costs 2000+ 40-byte DMA descriptors.
"""

import os
import numpy as np

N_CORES = 8
B_FULL = 16384
BS = B_FULL // N_CORES  # 2048 rows per core
IN_F = 784
HIC = 7                 # fc1 hi chunks of 112 rows (7*112 = 784)
HIR = 112
LOP = 4                 # fc1 lo DoubleRow pairs (8 chunks of 98 rows)
LOR = 98
XSC = 8192.0            # 2^13 pre-scale on both fc1 operand digits
H = 1024
HC = 8                  # hidden chunks of 128
OUT_F = 10
NSPLIT = 4              # batch column chunks of 512
NB = BS // NSPLIT       # 512
BT = BS // 128          # 16 batch tiles of 128 for the output transpose

LAST_RESULT = None      # BassKernelResults of the most recent run (for test.py)

_PLAN = {}


def _build_nc():
    import concourse.bass as bass
    import concourse.mybir as mybir
    import concourse.tile as tile
    from concourse.tile import add_dep_helper
    from concourse import bacc
    from concourse.bass import ts
    from concourse.masks import make_identity

    f32 = mybir.dt.float32
    f16 = mybir.dt.float16
    f8 = mybir.dt.float8e4
    AF = mybir.ActivationFunctionType
    ALU = mybir.AluOpType
    DR = mybir.MatmulPerfMode.DoubleRow

    nc = bacc.Bacc(None)

    xhi_t = nc.dram_tensor("xhi", [HIC, HIR, BS], f16, kind="ExternalInput")
    xlo_t = nc.dram_tensor("xlo", [LOP, LOR, 2, BS], f8, kind="ExternalInput")
    # weight layouts are host-pretransposed to [m, p, k, c] so every DMA is
    # a contiguous per-partition stream (no tiny strided descriptors)
    s1h_t = nc.dram_tensor("s1h", [HC, HIR, HIC, 128], f16, kind="ExternalInput")
    s1l_t = nc.dram_tensor("s1l", [HC, LOR, LOP, 2, 128], f8, kind="ExternalInput")
    s2_t = nc.dram_tensor("s2t", [HC, 128, HC, 128], f8, kind="ExternalInput")
    s3_t = nc.dram_tensor("s3t", [HC, 128, HC, 128], f8, kind="ExternalInput")
    w4_t = nc.dram_tensor("w4t", [128, HC, OUT_F], f16, kind="ExternalInput")
    b1_t = nc.dram_tensor("bias1", [128, HC], f32, kind="ExternalInput")
    b2_t = nc.dram_tensor("bias2", [128, HC], f32, kind="ExternalInput")
    sc3_t = nc.dram_tensor("sc3", [128, HC], f32, kind="ExternalInput")
    sh3_t = nc.dram_tensor("sh3", [128, HC], f32, kind="ExternalInput")
    b4_t = nc.dram_tensor("b4", [32, OUT_F], f32, kind="ExternalInput")
    # y stays in the SBUF-contiguous [32, blk, 10] layout; the host
    # un-permutes (free) instead of paying a 40B-per-row strided DMA
    y_t = nc.dram_tensor("y", [32, BS // 32, OUT_F], f32, kind="ExternalOutput")

    with tile.TileContext(nc) as tc:
        with (
            tc.tile_pool(name="consts", bufs=1) as consts,
            tc.tile_pool(name="tmp", bufs=4) as tmp,
            tc.tile_pool(name="psum", bufs=8, space="PSUM") as psum,
        ):
            junk16 = consts.tile([128, 512], f16, tag="junk16")
            xh_sb = consts.tile([HIR, HIC, BS], f16, tag="xh")
            xl_sb = consts.tile([LOR, LOP, 2, BS], f8, tag="xl")
            s1h_sb = consts.tile([HIR, HC, HIC, 128], f16, tag="s1h")
            s1l_sb = consts.tile([LOR, HC, LOP, 2, 128], f8, tag="s1l")
            s2_sb = consts.tile([128, HC, HC, 128], f8, tag="s2")
            s3_sb = consts.tile([128, HC, HC, 128], f8, tag="s3")
            w4_sb = consts.tile([128, HC, OUT_F], f16, tag="w4")
            b1v = consts.tile([128, HC], f32, tag="b1v")
            b2v = consts.tile([128, HC], f32, tag="b2v")
            sc3v = consts.tile([128, HC], f32, tag="sc3v")
            sh3v = consts.tile([128, HC], f32, tag="sh3v")
            b4v = consts.tile([32, OUT_F], f32, tag="b4v")
            act1 = consts.tile([128, HC, BS], f8, tag="act1")
            act2 = consts.tile([128, HC, BS], f8, tag="act2")
            act3 = consts.tile([128, HC, BS], f16, tag="act3")
            NBLK = BS // 32  # 64 batch blocks of 32 for the DVE transpose
            ltr = consts.tile([32, BS], f32, tag="ltr")
            es2 = consts.tile([32, NBLK, OUT_F], f32, tag="es2")
            lse2 = consts.tile([32, NBLK], f32, tag="lse2")
            outf2 = consts.tile([32, NBLK, OUT_F], f32, tag="outf2")
            u32 = mybir.dt.uint32
            lnmu = consts.tile([32, NBLK], u32, tag="lnmu")
            lnt = consts.tile([32, NBLK], f32, tag="lnt")
            lnbf = consts.tile([32, NBLK], f32, tag="lnbf")
            ksc = consts.tile([32, 1], f32, tag="ksc")
            kbi = consts.tile([32, 1], f32, tag="kbi")

            # ---- PE warmup: dummy matmuls on zeroed junk keep the PE busy
            # from the end of the NEFF preamble until real data arrives, so
            # the HAM clock-gate is at 8/8 (2.4 GHz) when fc1 starts and no
            # ramp time is wasted.
            # memset on gpsimd: it reaches body code ~0.8us before vector,
            # so the warmup (and the HAM SHORT window) starts that much sooner
            nc.gpsimd.memset(junk16, 0.0)
            wps = [psum.tile([128, NB], f32, tag="mm", name="wps") for _ in range(2)]
            # sized so the HAM SHORT window fires (needs a CONTIGUOUS ~3.4us
            # busy stretch covering a full free-running 4096-cycle window)
            # before the first data-paced stall can re-throttle the ramp
            for i in range(12):
                nc.tensor.matmul(wps[i % 2], junk16[:, 0:128], junk16,
                                 start=True, stop=True)
            for i in range(4):
                nc.tensor.matmul(wps[i % 2][:, 0:128], junk16[:, 0:128],
                                 junk16[:, 0:128], start=True, stop=True)

            # ---- input DMAs. Per-DMA-queue throughput is ~95-130GB/s
            # shared among in-flight transfers (2 HWDGE rings + 1 SWDGE,
            # ~260GB/s aggregate), so x streams in consumption order with
            # ~230KB granularity. Weight heads are split small so the
            # first LDWEIGHTS doesn't wait on a 200KB transfer.
            nc.sync.dma_start(out=s1h_sb[:, 0, 0:2], in_=s1h_t[0, :, 0:2])
            nc.scalar.dma_start(out=s1h_sb[:, 1, 0:2], in_=s1h_t[1, :, 0:2])
            nc.scalar.dma_start(out=b1v, in_=b1_t[:, :])
            # s1h chunk 2+ for m=0/1 ride SWDGE (needed only ~2 units in) so
            # the HW ring heads reach the first x bytes sooner
            nc.gpsimd.dma_start(out=s1h_sb[:, 0, 2:HIC], in_=s1h_t[0, :, 2:HIC])
            nc.gpsimd.dma_start(out=s1h_sb[:, 1, 2:HIC], in_=s1h_t[1, :, 2:HIC])
            # hi chunks as 230KB halves, one per HW ring per unit, in
            # consumption order; lo pair 0 rides the SWDGE ring (dep-free,
            # ahead of all dep-gated weight loads), pairs 1-3 the HW rings
            for u in range(HIC):
                nc.sync.dma_start(out=xh_sb[:, u, 0:NB * 2],
                                  in_=xhi_t[u, :, 0:NB * 2])
                nc.scalar.dma_start(out=xh_sb[:, u, NB * 2:BS],
                                    in_=xhi_t[u, :, NB * 2:BS])
            nc.gpsimd.dma_start(out=s1l_sb[:, 0], in_=s1l_t[0])
            nc.gpsimd.dma_start(out=s1l_sb[:, 1], in_=s1l_t[1])
            nc.gpsimd.dma_start(out=xl_sb[:, 0], in_=xlo_t[0])
            for p in (1, 2, 3):
                nc.sync.dma_start(out=xl_sb[:, p, :, 0:NB * 2],
                                  in_=xlo_t[p, :, :, 0:NB * 2])
                nc.scalar.dma_start(out=xl_sb[:, p, :, NB * 2:BS],
                                    in_=xlo_t[p, :, :, NB * 2:BS])
            # fc1 weights for m>=2: SWDGE, staggered behind fc1 progress
            s1_dmas = {}
            for m in range(2, HC):
                dh = nc.gpsimd.dma_start(out=s1h_sb[:, m], in_=s1h_t[m])
                dl = nc.gpsimd.dma_start(out=s1l_sb[:, m], in_=s1l_t[m])
                s1_dmas[m] = (dh, dl)

            # ---- fc1: h1 = x.T @ s1 (feature-major), sign -> act1 ----
            # units 0-6: fp16 hi chunks (112 rows); units 7-10: fp8
            # DoubleRow lo pairs (2x98 rows). All accumulate into one PSUM
            # group at 2^13 scale; sign ignores the scale (bias pre-scaled).
            # m=0 and m=1 interleave over units so every arriving x chunk
            # feeds 8 MMs -- the PE never queues a not-yet-arrived chunk's
            # MM ahead of runnable work during the x DMA window.
            NU = HIC + LOP  # 11 units
            psp = {m: [psum.tile([128, NB], f32, tag="mm", name="ps")
                       for _ in range(NSPLIT)] for m in (0, 1)}
            for u in range(NU):
                for m in (0, 1):
                    for n in range(NSPLIT):
                        if u < HIC:
                            nc.tensor.matmul(
                                psp[m][n], s1h_sb[:, m, u], xh_sb[:, u, ts(n, NB)],
                                start=(u == 0), stop=(u == NU - 1),
                            )
                        else:
                            p = u - HIC
                            nc.tensor.matmul(
                                psp[m][n], s1l_sb[:, m, p], xl_sb[:, p, :, ts(n, NB)],
                                start=False, stop=(u == NU - 1),
                                perf_mode=DR,
                            )
            for m in (0, 1):
                for n in range(NSPLIT):
                    a = nc.scalar.activation(
                        act1[:, m, ts(n, NB)], psp[m][n], AF.Sign, bias=b1v[:, m:m + 1]
                    )
                    if m == 1 and n == NSPLIT - 1:
                        x_done_gate = a
                    if n == 0 and m + 3 in s1_dmas:
                        for d in s1_dmas[m + 3]:
                            add_dep_helper(d.ins, a.ins, reason="stagger s1 loads")
            for m in range(2, HC):
                pss = [psum.tile([128, NB], f32, tag="mm", name="ps") for _ in range(NSPLIT)]
                for u in range(NU):
                    for n in range(NSPLIT):
                        if u < HIC:
                            nc.tensor.matmul(
                                pss[n], s1h_sb[:, m, u], xh_sb[:, u, ts(n, NB)],
                                start=(u == 0), stop=(u == NU - 1),
                            )
                        else:
                            p = u - HIC
                            nc.tensor.matmul(
                                pss[n], s1l_sb[:, m, p], xl_sb[:, p, :, ts(n, NB)],
                                start=False, stop=(u == NU - 1),
                                perf_mode=DR,
                            )
                for n in range(NSPLIT):
                    a = nc.scalar.activation(
                        act1[:, m, ts(n, NB)], pss[n], AF.Sign, bias=b1v[:, m:m + 1]
                    )
                    # stagger s1 weight loads two m-tiles ahead of use
                    if n == 0 and m + 3 in s1_dmas:
                        for d in s1_dmas[m + 3]:
                            add_dep_helper(d.ins, a.ins, reason="stagger s1 loads")

            # later-layer weights: gated behind fc1 m=1 so their transfers
            # don't steal HBM bandwidth from the x load during the ramp.
            # s2 rides the (now idle) sync HWDGE ring; s3 the SWDGE ring.
            for m in range(HC):
                d = nc.sync.dma_start(out=s2_sb[:, m], in_=s2_t[m])
                add_dep_helper(d.ins, x_done_gate.ins, reason="defer s2 after x load")
            nc.scalar.dma_start(out=b2v, in_=b2_t[:, :])
            for m in range(HC):
                d = nc.gpsimd.dma_start(out=s3_sb[:, m], in_=s3_t[m])
                add_dep_helper(d.ins, x_done_gate.ins, reason="defer s3 after x load")
            nc.scalar.dma_start(out=sc3v, in_=sc3_t[:, :])
            nc.scalar.dma_start(out=sh3v, in_=sh3_t[:, :])
            nc.scalar.dma_start(out=w4_sb, in_=w4_t[:, :, :])
            nc.scalar.dma_start(out=b4v, in_=b4_t[:, :])
            nc.vector.memset(ksc, 8.262958294867817e-08)    # ln2 * 2^-23
            nc.vector.memset(kbi, -88.81559010855409)       # -127*ln2 + ln2*c0

            # ---- fc2: binary x binary, fp8 DoubleRow, sign -> act2 ----
            # half-tiles (2 psum banks per group): the ScalarE psum-drain
            # vents every 2 n-chunks so bank recycling never stalls the PE
            for m in range(HC):
                for hf in range(2):
                    pss = [psum.tile([128, NB], f32, tag="mm", name="ps")
                           for _ in range(2)]
                    for kk in range(HC // 2):
                        ksl = slice(2 * kk, 2 * kk + 2)
                        for j in range(2):
                            n = 2 * hf + j
                            nc.tensor.matmul(
                                pss[j], s2_sb[:, m, ksl], act1[:, ksl, ts(n, NB)],
                                start=(kk == 0), stop=(kk == HC // 2 - 1),
                                perf_mode=DR,
                            )
                    for j in range(2):
                        n = 2 * hf + j
                        nc.scalar.activation(
                            act2[:, m, ts(n, NB)], pss[j], AF.Sign,
                            bias=b2v[:, m:m + 1]
                        )

            # ---- fc3: fp8 DoubleRow, bn affine + hardtanh -> act3 (DVE) ----
            for m in range(HC):
                for hf in range(2):
                    pss = [psum.tile([128, NB], f32, tag="mm", name="ps")
                           for _ in range(2)]
                    for kk in range(HC // 2):
                        ksl = slice(2 * kk, 2 * kk + 2)
                        for j in range(2):
                            n = 2 * hf + j
                            nc.tensor.matmul(
                                pss[j], s3_sb[:, m, ksl], act2[:, ksl, ts(n, NB)],
                                start=(kk == 0), stop=(kk == HC // 2 - 1),
                                perf_mode=DR,
                            )
                    for j in range(2):
                        n = 2 * hf + j
                        t = tmp.tile([128, NB], f32, tag="t3")
                        nc.scalar.activation(
                            t, pss[j], AF.Identity,
                            bias=sh3v[:, m:m + 1], scale=sc3v[:, m:m + 1],
                        )
                        nc.vector.tensor_scalar(
                            out=act3[:, m, ts(n, NB)], in0=t,
                            scalar1=-1.0, scalar2=1.0,
                            op0=ALU.max, op1=ALU.min,
                        )

            # ---- fc4 + log_softmax + store, streamed per 512-col n-chunk
            # so only the last chunk's (short) chain trails the final MM.
            # ltr[p, 32j+q] = logit class q of batch row 32j+p  (q < 10)
            # chunk widths: the final pieces are small so the post-last-MM
            # chain (copy/transpose/bias/exp/reduce) is short
            C3, C2, C1 = 0.10668443736693474, -0.7135874857296783, 1.3937265161635166

            def fin(bs, be, ring):
                # table-free ln(lse) for 32-blocks [bs:be), then subtract and
                # store those blocks. ln(s) = ln2*(float(bits)*2^-23 - 127
                # + c0) + poly3(m), m = mantissa in [1,2); dodges the 1.28us
                # Ln ACT_TABLE_LOAD (Ln is outside the boot-resident group).
                bsl = slice(bs, be)
                bits = lse2[:, bsl].bitcast(u32)
                nc.scalar.activation(lnbf[:, bsl], bits, AF.Identity,
                                     scale=ksc[:, :], bias=kbi[:, :])
                nc.vector.tensor_scalar(out=lnmu[:, bsl], in0=bits,
                                        scalar1=0x007FFFFF, op0=ALU.bitwise_and,
                                        scalar2=0x3F800000, op1=ALU.bitwise_or)
                mant = lnmu[:, bsl].bitcast(f32)
                nc.vector.tensor_scalar(out=lnt[:, bsl], in0=mant, scalar1=C3,
                                        op0=ALU.mult, scalar2=C2, op1=ALU.add)
                nc.vector.tensor_tensor(out=lnt[:, bsl], in0=lnt[:, bsl],
                                        in1=mant, op=ALU.mult)
                nc.vector.tensor_scalar(out=lnt[:, bsl], in0=lnt[:, bsl],
                                        scalar1=C1, op0=ALU.add, scalar2=None)
                nc.vector.tensor_tensor(out=lnt[:, bsl], in0=lnt[:, bsl],
                                        in1=mant, op=ALU.mult)
                nc.vector.tensor_tensor(out=lse2[:, bsl], in0=lnt[:, bsl],
                                        in1=lnbf[:, bsl], op=ALU.add)
                ltf = ltr[:, bs * 32:be * 32]
                ltvf = bass.AP(tensor=ltf.tensor, offset=ltf.offset,
                               ap=[ltf.ap[0], [32, be - bs], [1, OUT_F]])
                lser = lse2[:, bsl]
                nc.vector.tensor_tensor(
                    out=outf2[:, bsl], in0=ltvf,
                    in1=bass.AP(tensor=lser.tensor, offset=lser.offset,
                                ap=[lser.ap[0], lser.ap[1], [0, OUT_F]]),
                    op=ALU.subtract,
                )
                ring.dma_start(out=y_t[:, bsl], in_=outf2[:, bsl])

            pieces = [(0, NB), (NB, NB), (2 * NB, NB), (3 * NB, NB // 2),
                      (3 * NB + NB // 2, NB // 4), (3 * NB + 3 * NB // 4, NB // 4)]
            for pi, (st, w) in enumerate(pieces):
                wb = w // 32
                sb = st // 32
                csl = slice(st, st + w)
                bsl = slice(sb, sb + wb)
                # psum [32, w] so the DVE transpose can read it directly
                # (rows 10-31 junk, never consumed); b4 added post-transpose
                # on DVE (hidden engine slack) instead of costing PE columns
                ps4 = psum.tile([32, w], f32, tag="mm", name="ps4")
                for k in range(HC):
                    nc.tensor.matmul(
                        ps4[0:OUT_F, :], w4_sb[:, k], act3[:, k, csl],
                        start=(k == 0), stop=(k == HC - 1),
                    )
                nc.vector.transpose(ltr[:, csl], ps4[:, :])
                ltrn = ltr[:, csl]
                ltv = bass.AP(tensor=ltrn.tensor, offset=ltrn.offset,
                              ap=[ltrn.ap[0], [32, wb], [1, OUT_F]])
                b4r = b4v[:, :]
                nc.vector.tensor_tensor(
                    out=ltv, in0=ltv,
                    in1=bass.AP(tensor=b4r.tensor, offset=b4r.offset,
                                ap=[b4r.ap[0], [0, wb], b4r.ap[1]]),
                    op=ALU.add,
                )
                nc.scalar.activation(es2[:, bsl], ltv, AF.Exp)
                nc.vector.tensor_reduce(
                    out=lse2[:, bsl], in_=es2[:, bsl],
                    axis=mybir.AxisListType.X, op=ALU.add,
                )
                # finalize blocks 0-47 while pieces 3-5 are still on the PE,
                # blocks 48-59 during piece 5; only 4 blocks trail the last MM
                if pi == 2:
                    fin(0, 48, nc.scalar)
                elif pi == 4:
                    fin(48, 60, nc.scalar)
            fin(60, NBLK, nc.sync)

    nc.finalize()
    return nc


def _host_prep(inputs):
    """Shard x, binarize/lay out weights, fold bn into sign biases."""
    import ml_dtypes

    f16 = np.float16
    f8 = ml_dtypes.float8_e4m3

    x = np.asarray(inputs["x"], np.float32)
    w1 = np.asarray(inputs["w1"], np.float32)
    w2 = np.asarray(inputs["w2"], np.float32)
    w3 = np.asarray(inputs["w3"], np.float32)
    w4 = np.asarray(inputs["w4"], np.float32)
    b1 = np.asarray(inputs["b1"], np.float32)
    b2 = np.asarray(inputs["b2"], np.float32)
    b3 = np.asarray(inputs["b3"], np.float32)
    b4 = np.asarray(inputs["b4"], np.float32)

    EPS = np.float64(1e-5)

    def gv(i):
        return (np.asarray(inputs[f"g{i}"], np.float32),
                np.asarray(inputs[f"be{i}"], np.float32),
                np.asarray(inputs[f"m{i}"], np.float32),
                np.asarray(inputs[f"v{i}"], np.float32))

    g1, be1, m1, v1 = gv(1)
    g2, be2, m2, v2 = gv(2)
    g3, be3, m3, v3 = gv(3)
    # sign(bn(h)) == sign(h + (b - m)) requires gamma > 0 and beta == 0
    assert np.all(g1 > 0) and np.all(be1 == 0), "unsupported bn1 params"
    assert np.all(g2 > 0) and np.all(be2 == 0), "unsupported bn2 params"

    def pvec(v):  # [H] -> [128, HC] partition-major
        return np.ascontiguousarray(v.astype(np.float32).reshape(HC, 128).T)

    bias1 = pvec(b1 - m1) * np.float32(XSC)  # fc1 psum carries 2^13 * h1
    bias2 = pvec(b2 - m2)
    r3 = 1.0 / np.sqrt(v3.astype(np.float64) + EPS)
    sc3 = pvec(r3 * g3)
    sh3 = pvec((b3 - m3).astype(np.float64) * r3 * g3 + be3)

    def wlay(w, kc, dt):  # [out, in] -> [m, 128p(in), k, 128c(out)]
        st = np.sign(w).T.astype(np.float32)            # [in, out]
        kin = kc * 128
        if st.shape[0] < kin:
            st = np.pad(st, ((0, kin - st.shape[0]), (0, 0)))
        mo = st.shape[1] // 128
        return np.ascontiguousarray(
            st.reshape(kc, 128, mo, 128).transpose(2, 1, 0, 3)
        ).astype(dt)

    s1f = np.sign(w1).T.astype(np.float32)              # [784, 1024]
    # hi: [8m, 112, 7, 128] fp16; lo: [8m, 98, 4, 2, 128] fp8 (DR pairs)
    s1h = np.ascontiguousarray(
        s1f.reshape(HIC, HIR, HC, 128).transpose(2, 1, 0, 3)
    ).astype(f16)
    s1l = np.ascontiguousarray(
        s1f.reshape(2 * LOP, LOR, HC, 128).transpose(2, 1, 0, 3)
        .reshape(HC, LOR, LOP, 2, 128)
    ).astype(f8)
    s2t = wlay(w2, HC, f8)
    s3t = wlay(w3, HC, f8)
    w4t = np.ascontiguousarray(
        w4.T.astype(f16).reshape(HC, 128, OUT_F).transpose(1, 0, 2)
    )

    b4p = np.ascontiguousarray(np.broadcast_to(b4.reshape(1, OUT_F), (32, OUT_F)).astype(np.float32))
    shared = dict(s1h=s1h, s1l=s1l, s2t=s2t, s3t=s3t, w4t=w4t,
                  bias1=bias1, bias2=bias2, sc3=sc3, sh3=sh3, b4=b4p)
    in_maps = []
    for c in range(N_CORES):
        xs = x[c * BS:(c + 1) * BS]                     # [2048, 784]
        xt = xs.T                                       # [784, 2048]
        xhi = xt.astype(f16)
        xlo = xt - xhi.astype(np.float32)
        # both digits pre-scaled by 2^13 (exact): hi stays fp16-exact
        # (max |x|*8192 ~ 45k < 65504), lo clears the e4m3 subnormal floor
        xhi_s = (xhi.astype(np.float32) * np.float32(XSC)).astype(f16)
        xlo_s = (xlo * np.float32(XSC)).astype(f8)
        m = dict(shared)
        m["xhi"] = np.ascontiguousarray(xhi_s.reshape(HIC, HIR, BS))
        m["xlo"] = np.ascontiguousarray(
            xlo_s.reshape(LOP, 2, LOR, BS).transpose(0, 2, 1, 3))
        in_maps.append(m)
    return in_maps


def kernel(**inputs):
    global LAST_RESULT
    from concourse.bass_utils import run_bass_kernel_spmd

    if "nc" not in _PLAN:
        _PLAN["nc"] = _build_nc()
    nc = _PLAN["nc"]

    in_maps = _host_prep(inputs)
    br = run_bass_kernel_spmd(
        nc, in_maps, list(range(N_CORES)),
        tmpdir=os.environ.get("KERNEL_TMPDIR") or None,
    )
    LAST_RESULT = br
    out = np.concatenate(
        [br.results[c]["y"].transpose(1, 0, 2).reshape(BS, OUT_F)
         for c in range(N_CORES)], axis=0)
    return out.astype(np.float32)
